# revision 34
# baseline (speedup 1.0000x reference)
"""Trainium2 Bass kernel for DiffAttention (nn_DiffAttention_49847390437777).

Contract: kernel(**full_inputs) -> full output [2, 2048, 8, 256] fp32.

Sharding (8 cores): core c handles batch b = c//4 and global query-head pairs
{2*(c%4), 2*(c%4)+1} (i.e. heads 4*(c%4)..4*(c%4)+3).  Diff-attention couples
only adjacent head pairs, which stay co-located.  subln_weight is applied on
host after the gather (it multiplies AFTER the RMS norm, so this is exact).

Device algorithm per core (4 heads = 2 pairs, seq 2048, head_dim 128), all
bf16 matmul inputs (numpy study: all-bf16 rel err ~0.008 vs gate 2e-2; the
old fp32 first-superblock path was unnecessary):
  - scores transposed: S^T[k, q] = kT_blk.T @ qT_blk (contraction d=128 on
    partitions), causal blocks only; softmax without max-subtraction so the
    row-sum fuses into the PV matmul via an extra column on V.
  - exp on ACT per 2-kb group [128, 2, 512] (per-kb trimmed in the diagonal
    region); causal diagonal 128x128 blocks masked with a triangular tile on
    GpSimd (DVE for the first two steps, where GpSimd latency is exposed).
  - PV per 128-row j-block: par0 streams [v1|v2|c0], par1 streams
    [-v1|-v2|c1].  The bf16 V bytes are shared (negated) between the two
    heads so quantization noise cancels in the subtraction (an independently
    rounded lambda*V copy costs 10x accuracy - measured).  c0, c1 is a
    host-searched bf16 pair with c0/c1 ~= lambda to ~1e-5, so the
    normalize-and-subtract needs NO lambda multiply on device:
      A1' = O1 * recip(c0*rowsum1)                   (recip + tensor_scalar)
      diff' = (O2neg * recip(c1*rowsum2)) + A1'      (recip + one STT)
            = (A1 - lambda*A2)/c0
    The 1/c0 scale folds exactly into the RMS norm: eps' = eps/c0^2 is
    shipped via the aux tensor, and rms = exp(-0.5*ln(ssq/256 + eps') +
    ln(1-lambda_init)) then reproduces the reference output bit-for-near.
    Exp+Ln pinned to the one ACT table set holding both (no table thrash).
  - ssq via tensor_mul + reduce_sum, emitted one j late so they queue behind
    the next j's recip chain on the in-order DVE FIFO (PSUM recycling for
    the PV accumulators is gated by that chain).
  - flat software pipeline over (pair, qb) steps crossing the pair boundary:
    scores/exp/mask of step s+1 are emitted before PV+epilogue of step s, so
    exp(pair1,qb0) hides under PV(pair0,qb3).
  - per-half-qb rms/out-scale/store (bf16 output, un-cast on host) so the
    tail only waits on the last two j-blocks; loads and stores ride the
    otherwise-idle sync queue, loads chunked in need-order so the first QK
    starts ~1.5us after DMA go and PV(qb0) is not gated by the V load.
"""

import math
import os

import numpy as np
import ml_dtypes

HEAD_DIM = 128
N_HEADS = 16
LAYER_IDX = 12
LAMBDA_INIT = 0.8 - 0.6 * math.exp(-0.3 * (LAYER_IDX - 1))
EPS = 1e-5
SCALE = 1.0 / math.sqrt(HEAD_DIM)
S_FOLD = 1.0 - LAMBDA_INIT

B = 2
S = 2048
NB = S // 128   # 16 key blocks of 128
QB = S // 512   # 4 query superblocks of 512
N_CORES = 8

bf16 = ml_dtypes.bfloat16

_CACHE = {}
last_results = None  # BassKernelResults of the most recent run (for test.py)


def build_nc():
    """Build + compile the per-core Bass program (same program on all cores)."""
    import concourse.bass as bass
    import concourse.mybir as mybir
    import concourse.bacc as bacc
    import concourse.tile as tile
    from concourse.masks import make_upper_triangular
    from contextlib import ExitStack

    f32 = mybir.dt.float32
    b16 = mybir.dt.bfloat16
    AF = mybir.ActivationFunctionType
    ALU = mybir.AluOpType

    nc = bacc.Bacc("TRN2", target_bir_lowering=False, debug=False)

    # qkb layout per (pair, par): [0:512]=kT[:,0:512], [512:1024]=qT[:,0:512],
    # [1024:1536]=kT[:,512:1024], [1536:2048]=qT[:,512:1024],
    # [2048:3072]=kT[:,1024:2048], [3072:4096]=qT[:,1024:2048];
    # DMA'd in need-order: qb0-chunk, vx par0, qb1-chunk, vx par1, rest.
    # vxb per par: par0=[v1|v2|c0], par1=[-v1|-v2|c1] (see module docstring);
    # aux col0 carries eps/c0^2 for the rms fold.
    qkb = nc.dram_tensor("qkb", [2, 2, 128, 4096], b16, kind="ExternalInput")
    vxb = nc.dram_tensor("vxb", [2, 2, 128, NB, 257], b16, kind="ExternalInput")
    aux = nc.dram_tensor("aux", [128, 2], f32, kind="ExternalInput")
    o = nc.dram_tensor("o", [2, QB, 128, 4, 256], b16, kind="ExternalOutput")

    with tile.TileContext(nc) as tc:
        with ExitStack() as ctx:
            ec = ctx.enter_context
            const = ec(tc.tile_pool(name="const", bufs=1))
            qkpool = ec(tc.tile_pool(name="qkpool", bufs=2))
            vpool = ec(tc.tile_pool(name="vpool", bufs=2))
            ppool = ec(tc.tile_pool(name="ppool", bufs=2))
            apool = ec(tc.tile_pool(name="apool", bufs=2))
            dpool = ec(tc.tile_pool(name="dpool", bufs=2))
            stat = ec(tc.tile_pool(name="stat", bufs=3))
            tmp = ec(tc.tile_pool(name="tmp", bufs=4))
            opool = ec(tc.tile_pool(name="opool", bufs=2))
            spsum = ec(tc.tile_pool(name="spsum", bufs=2, space="PSUM"))
            opsum = ec(tc.tile_pool(name="opsum", bufs=2, space="PSUM"))

            tri16 = const.tile([128, 128], b16)
            make_upper_triangular(nc, tri16[:], val=1.0, diag=True)
            lsf_t = const.tile([128, 1], f32)
            nc.gpsimd.memset(lsf_t[:], math.log(S_FOLD))

            # loads all on the sync queue, in need-order; stores also on sync
            # (it is otherwise idle, and load issues all drain up-front)
            pairdat = {}
            for pair in range(2):
                qk = {}
                for par in range(2):
                    qk[par] = qkpool.tile([128, 4096], b16, tag=f"qk{par}",
                                          name=f"qk{par}")
                vx_b = vpool.tile([128, 2, NB, 257], b16, tag="vx", name="vx")
                # pair0's first chunks ride the ACT queue: its DGE issues
                # immediately (the table load isn't needed until ~11us)
                # while the sync queue is still warming up
                for par in range(2):
                    (nc.scalar if pair == 0 else nc.sync).dma_start(
                        qk[par][:, 0:1024], qkb[pair, par, :, 0:1024])
                nc.sync.dma_start(vx_b[:, 0], vxb[pair, 0])
                nc.sync.dma_start(qk[0][:, 1024:2048],
                                  qkb[pair, 0, :, 1024:2048])
                nc.sync.dma_start(vx_b[:, 1], vxb[pair, 1])
                nc.sync.dma_start(qk[1][:, 1024:2048],
                                  qkb[pair, 1, :, 1024:2048])
                for par in range(2):
                    nc.sync.dma_start(qk[par][:, 2048:4096],
                                      qkb[pair, par, :, 2048:4096])
                pairdat[pair] = (qk, vx_b)
            aux_t = const.tile([128, 2], f32)
            nc.sync.dma_start(aux_t[:], aux[:])
            eps_t = aux_t[:, 0:1]  # eps/c0^2 (c0 folds out in the rms)

            def kt_ap(qk, par, kb):
                t = qk[par]
                if kb < 4:
                    return t[:, kb * 128:(kb + 1) * 128]
                if kb < 8:
                    return t[:, 1024 + (kb - 4) * 128:1024 + (kb - 3) * 128]
                return t[:, 2048 + (kb - 8) * 128:2048 + (kb - 7) * 128]

            def qt_ap(qk, par, qb, qoff):
                t = qk[par]
                if qb == 0:
                    return t[:, 512 + qoff:1024]
                if qb == 1:
                    return t[:, 1536 + qoff:2048]
                base = 3072 + (qb - 2) * 512
                return t[:, base + qoff:base + 512]

            # flat software pipeline over (pair, qb) steps, crossing the pair
            # boundary: scores/exp/mask for step s+1 are emitted before the
            # PV+epilogue of step s, so exp(pair1,qb0) hides under PV(pair0,qb3)
            prev = None  # (pair, qb, {par: pt tile}, vx_b)
            for step in range(2 * QB + 1):
                if step < 2 * QB:
                    pair, qb = divmod(step, QB)
                    qk, vx_b = pairdat[pair]
                    nkb = 4 * qb + 4
                    cur = {}
                    for par in range(2):
                        p1 = ppool.tile([128, NB, 512], b16,
                                        tag=f"pt{par}", name=f"pt{par}")
                        cur[par] = p1
                        for g in range(nkb // 2):
                            sp = spsum.tile([128, 2, 512], f32, tag="sp")
                            for t in range(2):
                                kb = 2 * g + t
                                qoff = max(0, (kb - 4 * qb)) * 128
                                nc.tensor.matmul(
                                    sp[:, t, qoff:512],
                                    kt_ap(qk, par, kb),
                                    qt_ap(qk, par, qb, qoff),
                                    start=True, stop=True,
                                )
                            if 2 * g + 1 < 4 * qb:
                                nc.scalar.activation(
                                    p1[:, 2 * g:2 * g + 2, :], sp[:, :, :],
                                    AF.Exp, scale=SCALE,
                                )
                            else:
                                for t in range(2):
                                    kb = 2 * g + t
                                    qoff = max(0, (kb - 4 * qb)) * 128
                                    nc.scalar.activation(
                                        p1[:, kb, qoff:512],
                                        sp[:, t, qoff:512],
                                        AF.Exp, scale=SCALE,
                                    )
                            mask_eng = (nc.vector if step <= 1 or step == QB
                                        else nc.gpsimd)
                            for t in range(2):
                                kb = 2 * g + t
                                if kb >= 4 * qb:
                                    qoff = (kb - 4 * qb) * 128
                                    mask_eng.tensor_mul(
                                        p1[:, kb, qoff:qoff + 128],
                                        p1[:, kb, qoff:qoff + 128],
                                        tri16[:],
                                    )
                    nxt = (pair, qb, cur, vx_b)
                else:
                    nxt = None
                if prev is not None:
                    ppair, pqb, ppt, pvx = prev
                    A1q = apool.tile([128, 4, 256], f32, tag="A1", name="A1q")
                    diffq = dpool.tile([128, 4, 256], b16, tag="diff",
                                       name="diffq")
                    ssq = stat.tile([128, 4], f32, tag="ssq", name="ssq")
                    rmst = stat.tile([128, 4], f32, tag="rms", name="rmst")
                    otile = opool.tile([128, 4, 256], b16, tag="ot",
                                       name="otile")
                    def emit_sqred(jj):
                        sqt = tmp.tile([128, 256], b16, tag="sqt")
                        nc.vector.tensor_mul(
                            sqt[:], diffq[:, jj, :], diffq[:, jj, :])
                        nc.vector.reduce_sum(
                            ssq[:, jj:jj + 1], sqt[:],
                            axis=mybir.AxisListType.X)

                    def emit_rms(jj):
                        h0 = jj - 1
                        lnm = stat.tile([128, 2], f32, tag="lnm", name="lnm")
                        nc.scalar.activation(
                            lnm[:], ssq[:, h0:jj + 1], AF.Ln,
                            scale=1.0 / 256.0, bias=eps_t[:])
                        nc.scalar.activation(
                            rmst[:, h0:jj + 1], lnm[:], AF.Exp,
                            scale=-0.5, bias=lsf_t[:])
                        for ji in (h0, jj):
                            nc.vector.tensor_scalar_mul(
                                otile[:, ji, :], diffq[:, ji, :],
                                rmst[:, ji:ji + 1])
                        nc.sync.dma_start(
                            o[ppair, pqb, :, h0:jj + 1],
                            otile[:, h0:jj + 1, :])

                    pend = None
                    for j in range(4):
                        jabs = 4 * pqb + j
                        ops = {}
                        for par in (0, 1):
                            op_t = opsum.tile([128, 257], f32,
                                              tag=f"op{par}")
                            ops[par] = op_t
                            for kb in range(jabs + 1):
                                nc.tensor.matmul(
                                    op_t[:],
                                    ppt[par][:, kb, j * 128:(j + 1) * 128],
                                    pvx[:, par, kb, :],
                                    start=(kb == 0), stop=(kb == jabs),
                                )
                            if par == 0:
                                rc1 = tmp.tile([128, 1], f32, tag="rc1")
                                nc.vector.reciprocal(
                                    rc1[:], op_t[:, 256:257])
                                nc.vector.tensor_scalar_mul(
                                    A1q[:, j, :], op_t[:, 0:256], rc1[:])
                        op_t = ops[1]
                        # par1 streams [-v1|-v2|c1] with c0/c1 ~ lambda to
                        # ~1e-5 (host-searched bf16 pair), so the fused
                        # normalize-and-subtract needs no lambda multiply:
                        # diff' = O1/(c0 r1) - O2/(c1 r2) = (A1 - lam*A2)/c0
                        rcl = tmp.tile([128, 1], f32, tag="rcl")
                        nc.vector.reciprocal(rcl[:], op_t[:, 256:257])
                        nc.vector.scalar_tensor_tensor(
                            diffq[:, j, :], op_t[:, 0:256], rcl[:],
                            A1q[:, j, :], ALU.mult, ALU.add)
                        # sq/reduce of the PREVIOUS j: behind this j's recip
                        # chain in the DVE FIFO, so PSUM recycling isn't
                        # delayed; rms fires once its pair of ssq is in
                        if step == 2 * QB:
                            emit_sqred(j)
                            if j in (1, 3):
                                emit_rms(j)
                        else:
                            if pend is not None:
                                emit_sqred(pend)
                                if pend == 1:
                                    emit_rms(1)
                            pend = j
                    if step != 2 * QB:
                        emit_sqred(3)
                        emit_rms(3)
                prev = nxt

    # Pin Exp+Ln to the one table set containing both
    # (natural_log_exp_and_others) — the greedy per-function chooser otherwise
    # thrashes between exp_and_others and the ln set (~1.3us per reload, and it
    # serializes the pipeline around each switch).
    AF = mybir.ActivationFunctionType
    _orig_gat = bacc.get_activation_tables

    def _gat(arch):
        tabs = _orig_gat(arch)
        for name, fns in tabs.items():
            if name != "natural_log_exp_and_others":
                fns.discard(AF.Exp)
                fns.discard(AF.Ln)
        return tabs

    bacc.get_activation_tables = _gat
    try:
        nc.compile()
    finally:
        bacc.get_activation_tables = _orig_gat
    return nc


def _find_c0c1(lam):
    """bf16 pair (c0, c1) with c0/c1 ~= lam to ~1e-5 (both exactly
    representable, so the ones-columns carry lambda with no bf16 bias)."""
    best = None
    for m in range(256):
        c0 = float(np.float32(bf16(0.5 * (1.0 + m / 256.0))))
        c1 = float(np.float32(bf16(c0 / lam)))
        if c1 <= 0:
            continue
        bias = abs(c0 / (c1 * lam) - 1.0)
        if best is None or bias < best[0]:
            best = (bias, c0, c1)
    return best[1], best[2]


def _prep_core_inputs(q, k, v, lam_full):
    """Host-side shard + layout prep. Returns list of 8 per-core input dicts."""
    c0, c1 = _find_c0c1(float(lam_full))
    aux_ = np.zeros((128, 2), np.float32)
    aux_[:, 0] = EPS / (c0 * c0)
    in_maps = []
    for c in range(N_CORES):
        b = c // 4
        h0 = 4 * (c % 4)
        # [s, 4, d] -> [4, d, s]
        qs = np.ascontiguousarray(q[b, :, h0:h0 + 4, :].transpose(1, 2, 0))
        ks = np.ascontiguousarray(k[b, :, h0:h0 + 4, :].transpose(1, 2, 0))
        # qkb: [pair, par, p, k512|q512|k512|q512|k1024|q1024] bf16
        qkb_ = np.empty((2, 2, 128, 4096), bf16)
        for pair in range(2):
            for par in range(2):
                h = 2 * pair + par
                qkb_[pair, par, :, 0:512] = ks[h][:, 0:512].astype(bf16)
                qkb_[pair, par, :, 512:1024] = qs[h][:, 0:512].astype(bf16)
                qkb_[pair, par, :, 1024:1536] = ks[h][:, 512:1024].astype(bf16)
                qkb_[pair, par, :, 1536:2048] = qs[h][:, 512:1024].astype(bf16)
                qkb_[pair, par, :, 2048:3072] = ks[h][:, 1024:2048].astype(bf16)
                qkb_[pair, par, :, 3072:4096] = qs[h][:, 1024:2048].astype(bf16)
        # par0 = [v1|v2|c0]; par1 = [-v1|-v2|c1].  bf16(-x) == -bf16(x), so
        # the V quantization noise stays perfectly (anti-)correlated between
        # the two heads and cancels in the diff subtraction; c0/c1 carries
        # lambda with ~1e-5 bias since both are exactly representable.
        vx = np.empty((2, 2, S, 257), np.float32)
        for pair in range(2):
            v1 = v[b, :, h0 + 2 * pair, :]
            v2 = v[b, :, h0 + 2 * pair + 1, :]
            vx[pair, 0, :, :128] = v1
            vx[pair, 0, :, 128:256] = v2
            vx[pair, 0, :, 256] = c0
            vx[pair, 1, :, :128] = -v1
            vx[pair, 1, :, 128:256] = -v2
            vx[pair, 1, :, 256] = c1
        # [2, 2, s, 257] -> partition-major [2, 2, 128, nb, 257]
        vxp = vx.reshape(2, 2, NB, 128, 257).transpose(0, 1, 3, 2, 4)
        vxb_ = np.ascontiguousarray(vxp).astype(bf16)
        in_maps.append({"qkb": qkb_, "vxb": vxb_, "aux": aux_})
    return in_maps


def kernel(q, k, v, lambda_q1, lambda_k1, lambda_q2, lambda_k2,
           subln_weight, attention_mask):
    global last_results
    from concourse.bass_utils import run_bass_kernel_spmd

    q = np.ascontiguousarray(np.asarray(q, np.float32))
    k = np.ascontiguousarray(np.asarray(k, np.float32))
    v = np.ascontiguousarray(np.asarray(v, np.float32))
    lam1 = np.exp(np.sum(np.asarray(lambda_q1, np.float32)
                         * np.asarray(lambda_k1, np.float32), dtype=np.float32))
    lam2 = np.exp(np.sum(np.asarray(lambda_q2, np.float32)
                         * np.asarray(lambda_k2, np.float32), dtype=np.float32))
    lam_full = np.float32(lam1 - lam2 + np.float32(LAMBDA_INIT))

    if "nc" not in _CACHE:
        _CACHE["nc"] = build_nc()
    nc = _CACHE["nc"]

    in_maps = _prep_core_inputs(q, k, v, lam_full)
    trace = bool(int(os.environ.get("KERNEL_TRACE", "0")))
    kw = {}
    if trace:
        kw = dict(trace=True, trace_cores=list(range(N_CORES)))
    res = run_bass_kernel_spmd(nc, in_maps, core_ids=list(range(N_CORES)), **kw)
    last_results = res

    out = np.empty((B, S, N_HEADS // 2, 256), np.float32)
    for c in range(N_CORES):
        b = c // 4
        gp = 2 * (c % 4)
        # o: [pair, qb, 128, 4, 256] bf16; row s = qb*512 + j*128 + p
        oc = res.results[c]["o"].astype(np.float32)
        oc = oc.transpose(0, 1, 3, 2, 4).reshape(2, S, 256)
        out[b, :, gp, :] = oc[0]
        out[b, :, gp + 1, :] = oc[1]
    out *= np.asarray(subln_weight, np.float32)[None, None, None, :]
    return out


# revision 36
# speedup vs baseline: 1.0216x; 1.0216x over previous
"""Trainium2 Bass kernel for DiffAttention (nn_DiffAttention_49847390437777).

Contract: kernel(**full_inputs) -> full output [2, 2048, 8, 256] fp32.

Sharding (8 cores): core c handles batch b = c//4 and global query-head pairs
{2*(c%4), 2*(c%4)+1} (i.e. heads 4*(c%4)..4*(c%4)+3).  Diff-attention couples
only adjacent head pairs, which stay co-located.  subln_weight is applied on
host after the gather (it multiplies AFTER the RMS norm, so this is exact).

Device algorithm per core (4 heads = 2 pairs, seq 2048, head_dim 128), all
bf16 matmul inputs (numpy study: all-bf16 rel err ~0.008 vs gate 2e-2; the
old fp32 first-superblock path was unnecessary):
  - scores transposed: S^T[k, q] = kT_blk.T @ qT_blk (contraction d=128 on
    partitions), causal blocks only; softmax without max-subtraction so the
    row-sum fuses into the PV matmul via an extra column on V.
  - exp on ACT per 2-kb group [128, 2, 512] (per-kb trimmed in the diagonal
    region); causal diagonal 128x128 blocks masked with a triangular tile on
    GpSimd (DVE for the first two steps, where GpSimd latency is exposed).
  - PV per 128-row j-block: par0 streams [v1|v2|c0], par1 streams
    [-v1|-v2|c1].  The bf16 V bytes are shared (negated) between the two
    heads so quantization noise cancels in the subtraction (an independently
    rounded lambda*V copy costs 10x accuracy - measured).  c0, c1 is a
    host-searched bf16 pair with c0/c1 ~= lambda to ~1e-5, so the
    normalize-and-subtract needs NO lambda multiply on device:
      A1' = O1 * recip(c0*rowsum1)                   (recip + tensor_scalar)
      diff' = (O2neg * recip(c1*rowsum2)) + A1'      (recip + one STT)
            = (A1 - lambda*A2)/c0
    The 1/c0 scale folds exactly into the RMS norm: eps' = eps/c0^2 is
    shipped via the aux tensor, and rms = exp(-0.5*ln(ssq/256 + eps') +
    ln(1-lambda_init)) then reproduces the reference output bit-for-near.
    Exp+Ln pinned to the one ACT table set holding both (no table thrash).
  - ssq via tensor_mul + reduce_sum, emitted one j late so they queue behind
    the next j's recip chain on the in-order DVE FIFO (PSUM recycling for
    the PV accumulators is gated by that chain).
  - flat software pipeline over (pair, qb) steps crossing the pair boundary:
    scores/exp/mask of step s+1 are emitted before PV+epilogue of step s, so
    exp(pair1,qb0) hides under PV(pair0,qb3).
  - per-half-qb rms/out-scale/store (bf16 output, un-cast on host) so the
    tail only waits on the last two j-blocks; loads and stores ride the
    otherwise-idle sync queue, loads chunked in need-order so the first QK
    starts ~1.5us after DMA go and PV(qb0) is not gated by the V load.
"""

import math
import os

import numpy as np
import ml_dtypes

HEAD_DIM = 128
N_HEADS = 16
LAYER_IDX = 12
LAMBDA_INIT = 0.8 - 0.6 * math.exp(-0.3 * (LAYER_IDX - 1))
EPS = 1e-5
SCALE = 1.0 / math.sqrt(HEAD_DIM)
S_FOLD = 1.0 - LAMBDA_INIT

B = 2
S = 2048
NB = S // 128   # 16 key blocks of 128
QB = S // 512   # 4 query superblocks of 512
N_CORES = 8

bf16 = ml_dtypes.bfloat16

_CACHE = {}
last_results = None  # BassKernelResults of the most recent run (for test.py)


def build_nc():
    """Build + compile the per-core Bass program (same program on all cores)."""
    import concourse.bass as bass
    import concourse.mybir as mybir
    import concourse.bacc as bacc
    import concourse.tile as tile
    from concourse.masks import make_upper_triangular
    from contextlib import ExitStack

    f32 = mybir.dt.float32
    b16 = mybir.dt.bfloat16
    AF = mybir.ActivationFunctionType
    ALU = mybir.AluOpType

    nc = bacc.Bacc("TRN2", target_bir_lowering=False, debug=False)

    # qkb layout per (pair, par): [0:512]=kT[:,0:512], [512:1024]=qT[:,0:512],
    # [1024:1536]=kT[:,512:1024], [1536:2048]=qT[:,512:1024],
    # [2048:3072]=kT[:,1024:2048], [3072:4096]=qT[:,1024:2048];
    # DMA'd in need-order: qb0-chunk, vx par0, qb1-chunk, vx par1, rest.
    # vxb per par: par0=[v1|v2|c0], par1=[-v1|-v2|c1] (see module docstring);
    # aux col0 carries eps/c0^2 for the rms fold.
    qkb = nc.dram_tensor("qkb", [2, 2, 128, 4096], b16, kind="ExternalInput")
    vxb = nc.dram_tensor("vxb", [2, 2, 128, NB, 257], b16, kind="ExternalInput")
    aux = nc.dram_tensor("aux", [128, 2], f32, kind="ExternalInput")
    o = nc.dram_tensor("o", [2, QB, 128, 4, 256], b16, kind="ExternalOutput")

    with tile.TileContext(nc) as tc:
        with ExitStack() as ctx:
            ec = ctx.enter_context
            const = ec(tc.tile_pool(name="const", bufs=1))
            qkpool = ec(tc.tile_pool(name="qkpool", bufs=2))
            vpool = ec(tc.tile_pool(name="vpool", bufs=2))
            ppool = ec(tc.tile_pool(name="ppool", bufs=2))
            apool = ec(tc.tile_pool(name="apool", bufs=2))
            dpool = ec(tc.tile_pool(name="dpool", bufs=2))
            stat = ec(tc.tile_pool(name="stat", bufs=3))
            tmp = ec(tc.tile_pool(name="tmp", bufs=4))
            opool = ec(tc.tile_pool(name="opool", bufs=2))
            spsum = ec(tc.tile_pool(name="spsum", bufs=2, space="PSUM"))
            opsum = ec(tc.tile_pool(name="opsum", bufs=2, space="PSUM"))

            # pair0's first QK chunks issue from the gpsimd queue BEFORE the
            # constant setup: the sync DGE spends ~2us generating its first
            # descriptors, and gpsimd's own work (tri16) isn't needed until
            # the first mask at ~13us
            qk0 = {}
            for par in range(2):
                qk0[par] = qkpool.tile([128, 4096], b16, tag=f"qk{par}",
                                       name=f"qk{par}")
                nc.gpsimd.dma_start(qk0[par][:, 0:1024],
                                    qkb[0, par, :, 0:1024])

            tri16 = const.tile([128, 128], b16)
            make_upper_triangular(nc, tri16[:], val=1.0, diag=True)
            lsf_t = const.tile([128, 1], f32)
            nc.gpsimd.memset(lsf_t[:], math.log(S_FOLD))

            # loads all on the sync queue, in need-order; stores also on sync
            # (it is otherwise idle, and load issues all drain up-front)
            pairdat = {}
            for pair in range(2):
                if pair == 0:
                    qk = qk0
                else:
                    qk = {}
                    for par in range(2):
                        qk[par] = qkpool.tile([128, 4096], b16,
                                              tag=f"qk{par}", name=f"qk{par}")
                vx_b = vpool.tile([128, 2, NB, 257], b16, tag="vx", name="vx")
                if pair == 1:
                    for par in range(2):
                        nc.sync.dma_start(qk[par][:, 0:1024],
                                          qkb[pair, par, :, 0:1024])
                nc.sync.dma_start(vx_b[:, 0], vxb[pair, 0])
                nc.sync.dma_start(qk[0][:, 1024:2048],
                                  qkb[pair, 0, :, 1024:2048])
                nc.sync.dma_start(vx_b[:, 1], vxb[pair, 1])
                nc.sync.dma_start(qk[1][:, 1024:2048],
                                  qkb[pair, 1, :, 1024:2048])
                for par in range(2):
                    nc.sync.dma_start(qk[par][:, 2048:4096],
                                      qkb[pair, par, :, 2048:4096])
                pairdat[pair] = (qk, vx_b)
            aux_t = const.tile([128, 2], f32)
            nc.sync.dma_start(aux_t[:], aux[:])
            eps_t = aux_t[:, 0:1]  # eps/c0^2 (c0 folds out in the rms)

            def kt_ap(qk, par, kb):
                t = qk[par]
                if kb < 4:
                    return t[:, kb * 128:(kb + 1) * 128]
                if kb < 8:
                    return t[:, 1024 + (kb - 4) * 128:1024 + (kb - 3) * 128]
                return t[:, 2048 + (kb - 8) * 128:2048 + (kb - 7) * 128]

            def qt_ap(qk, par, qb, qoff):
                t = qk[par]
                if qb == 0:
                    return t[:, 512 + qoff:1024]
                if qb == 1:
                    return t[:, 1536 + qoff:2048]
                base = 3072 + (qb - 2) * 512
                return t[:, base + qoff:base + 512]

            # flat software pipeline over (pair, qb) steps, crossing the pair
            # boundary: scores/exp/mask for step s+1 are emitted before the
            # PV+epilogue of step s, so exp(pair1,qb0) hides under PV(pair0,qb3)
            prev = None  # (pair, qb, {par: pt tile}, vx_b)
            for step in range(2 * QB + 1):
                if step < 2 * QB:
                    pair, qb = divmod(step, QB)
                    qk, vx_b = pairdat[pair]
                    nkb = 4 * qb + 4
                    cur = {}
                    for par in range(2):
                        p1 = ppool.tile([128, NB, 512], b16,
                                        tag=f"pt{par}", name=f"pt{par}")
                        cur[par] = p1
                        for g in range(nkb // 2):
                            sp = spsum.tile([128, 2, 512], f32, tag="sp")
                            for t in range(2):
                                kb = 2 * g + t
                                qoff = max(0, (kb - 4 * qb)) * 128
                                nc.tensor.matmul(
                                    sp[:, t, qoff:512],
                                    kt_ap(qk, par, kb),
                                    qt_ap(qk, par, qb, qoff),
                                    start=True, stop=True,
                                )
                            if 2 * g + 1 < 4 * qb:
                                nc.scalar.activation(
                                    p1[:, 2 * g:2 * g + 2, :], sp[:, :, :],
                                    AF.Exp, scale=SCALE,
                                )
                            else:
                                for t in range(2):
                                    kb = 2 * g + t
                                    qoff = max(0, (kb - 4 * qb)) * 128
                                    nc.scalar.activation(
                                        p1[:, kb, qoff:512],
                                        sp[:, t, qoff:512],
                                        AF.Exp, scale=SCALE,
                                    )
                            mask_eng = nc.vector if step <= 1 else nc.gpsimd
                            for t in range(2):
                                kb = 2 * g + t
                                if kb >= 4 * qb:
                                    qoff = (kb - 4 * qb) * 128
                                    mask_eng.tensor_mul(
                                        p1[:, kb, qoff:qoff + 128],
                                        p1[:, kb, qoff:qoff + 128],
                                        tri16[:],
                                    )
                    nxt = (pair, qb, cur, vx_b)
                else:
                    nxt = None
                if prev is not None:
                    ppair, pqb, ppt, pvx = prev
                    A1q = apool.tile([128, 4, 256], f32, tag="A1", name="A1q")
                    diffq = dpool.tile([128, 4, 256], b16, tag="diff",
                                       name="diffq")
                    ssq = stat.tile([128, 4], f32, tag="ssq", name="ssq")
                    rmst = stat.tile([128, 4], f32, tag="rms", name="rmst")
                    otile = opool.tile([128, 4, 256], b16, tag="ot",
                                       name="otile")
                    def emit_sqred(jj):
                        sqt = tmp.tile([128, 256], b16, tag="sqt")
                        nc.vector.tensor_mul(
                            sqt[:], diffq[:, jj, :], diffq[:, jj, :])
                        nc.vector.reduce_sum(
                            ssq[:, jj:jj + 1], sqt[:],
                            axis=mybir.AxisListType.X)

                    def emit_rms(jj):
                        h0 = jj - 1
                        lnm = stat.tile([128, 2], f32, tag="lnm", name="lnm")
                        nc.scalar.activation(
                            lnm[:], ssq[:, h0:jj + 1], AF.Ln,
                            scale=1.0 / 256.0, bias=eps_t[:])
                        nc.scalar.activation(
                            rmst[:, h0:jj + 1], lnm[:], AF.Exp,
                            scale=-0.5, bias=lsf_t[:])
                        for ji in (h0, jj):
                            nc.vector.tensor_scalar_mul(
                                otile[:, ji, :], diffq[:, ji, :],
                                rmst[:, ji:ji + 1])
                        nc.sync.dma_start(
                            o[ppair, pqb, :, h0:jj + 1],
                            otile[:, h0:jj + 1, :])

                    pend = None
                    for j in range(4):
                        jabs = 4 * pqb + j
                        ops = {}
                        for par in (0, 1):
                            op_t = opsum.tile([128, 257], f32,
                                              tag=f"op{par}")
                            ops[par] = op_t
                            for kb in range(jabs + 1):
                                nc.tensor.matmul(
                                    op_t[:],
                                    ppt[par][:, kb, j * 128:(j + 1) * 128],
                                    pvx[:, par, kb, :],
                                    start=(kb == 0), stop=(kb == jabs),
                                )
                            if par == 0:
                                rc1 = tmp.tile([128, 1], f32, tag="rc1")
                                nc.vector.reciprocal(
                                    rc1[:], op_t[:, 256:257])
                                nc.vector.tensor_scalar_mul(
                                    A1q[:, j, :], op_t[:, 0:256], rc1[:])
                        op_t = ops[1]
                        # par1 streams [-v1|-v2|c1] with c0/c1 ~ lambda to
                        # ~1e-5 (host-searched bf16 pair), so the fused
                        # normalize-and-subtract needs no lambda multiply:
                        # diff' = O1/(c0 r1) - O2/(c1 r2) = (A1 - lam*A2)/c0
                        rcl = tmp.tile([128, 1], f32, tag="rcl")
                        nc.vector.reciprocal(rcl[:], op_t[:, 256:257])
                        nc.vector.scalar_tensor_tensor(
                            diffq[:, j, :], op_t[:, 0:256], rcl[:],
                            A1q[:, j, :], ALU.mult, ALU.add)
                        # sq/reduce of the PREVIOUS j: behind this j's recip
                        # chain in the DVE FIFO, so PSUM recycling isn't
                        # delayed; rms fires once its pair of ssq is in
                        if step == 2 * QB:
                            emit_sqred(j)
                            if j in (1, 3):
                                emit_rms(j)
                        else:
                            if pend is not None:
                                emit_sqred(pend)
                                if pend == 1:
                                    emit_rms(1)
                            pend = j
                    if step != 2 * QB:
                        emit_sqred(3)
                        emit_rms(3)
                prev = nxt

    # Pin Exp+Ln to the one table set containing both
    # (natural_log_exp_and_others) — the greedy per-function chooser otherwise
    # thrashes between exp_and_others and the ln set (~1.3us per reload, and it
    # serializes the pipeline around each switch).
    AF = mybir.ActivationFunctionType
    _orig_gat = bacc.get_activation_tables

    def _gat(arch):
        tabs = _orig_gat(arch)
        for name, fns in tabs.items():
            if name != "natural_log_exp_and_others":
                fns.discard(AF.Exp)
                fns.discard(AF.Ln)
        return tabs

    bacc.get_activation_tables = _gat
    try:
        nc.compile()
    finally:
        bacc.get_activation_tables = _orig_gat
    return nc


def _find_c0c1(lam):
    """bf16 pair (c0, c1) with c0/c1 ~= lam to ~1e-5 (both exactly
    representable, so the ones-columns carry lambda with no bf16 bias)."""
    best = None
    for m in range(256):
        c0 = float(np.float32(bf16(0.5 * (1.0 + m / 256.0))))
        c1 = float(np.float32(bf16(c0 / lam)))
        if c1 <= 0:
            continue
        bias = abs(c0 / (c1 * lam) - 1.0)
        if best is None or bias < best[0]:
            best = (bias, c0, c1)
    return best[1], best[2]


def _prep_core_inputs(q, k, v, lam_full):
    """Host-side shard + layout prep. Returns list of 8 per-core input dicts."""
    c0, c1 = _find_c0c1(float(lam_full))
    aux_ = np.zeros((128, 2), np.float32)
    aux_[:, 0] = EPS / (c0 * c0)
    in_maps = []
    for c in range(N_CORES):
        b = c // 4
        h0 = 4 * (c % 4)
        # [s, 4, d] -> [4, d, s]
        qs = np.ascontiguousarray(q[b, :, h0:h0 + 4, :].transpose(1, 2, 0))
        ks = np.ascontiguousarray(k[b, :, h0:h0 + 4, :].transpose(1, 2, 0))
        # qkb: [pair, par, p, k512|q512|k512|q512|k1024|q1024] bf16
        qkb_ = np.empty((2, 2, 128, 4096), bf16)
        for pair in range(2):
            for par in range(2):
                h = 2 * pair + par
                qkb_[pair, par, :, 0:512] = ks[h][:, 0:512].astype(bf16)
                qkb_[pair, par, :, 512:1024] = qs[h][:, 0:512].astype(bf16)
                qkb_[pair, par, :, 1024:1536] = ks[h][:, 512:1024].astype(bf16)
                qkb_[pair, par, :, 1536:2048] = qs[h][:, 512:1024].astype(bf16)
                qkb_[pair, par, :, 2048:3072] = ks[h][:, 1024:2048].astype(bf16)
                qkb_[pair, par, :, 3072:4096] = qs[h][:, 1024:2048].astype(bf16)
        # par0 = [v1|v2|c0]; par1 = [-v1|-v2|c1].  bf16(-x) == -bf16(x), so
        # the V quantization noise stays perfectly (anti-)correlated between
        # the two heads and cancels in the diff subtraction; c0/c1 carries
        # lambda with ~1e-5 bias since both are exactly representable.
        vx = np.empty((2, 2, S, 257), np.float32)
        for pair in range(2):
            v1 = v[b, :, h0 + 2 * pair, :]
            v2 = v[b, :, h0 + 2 * pair + 1, :]
            vx[pair, 0, :, :128] = v1
            vx[pair, 0, :, 128:256] = v2
            vx[pair, 0, :, 256] = c0
            vx[pair, 1, :, :128] = -v1
            vx[pair, 1, :, 128:256] = -v2
            vx[pair, 1, :, 256] = c1
        # [2, 2, s, 257] -> partition-major [2, 2, 128, nb, 257]
        vxp = vx.reshape(2, 2, NB, 128, 257).transpose(0, 1, 3, 2, 4)
        vxb_ = np.ascontiguousarray(vxp).astype(bf16)
        in_maps.append({"qkb": qkb_, "vxb": vxb_, "aux": aux_})
    return in_maps


def kernel(q, k, v, lambda_q1, lambda_k1, lambda_q2, lambda_k2,
           subln_weight, attention_mask):
    global last_results
    from concourse.bass_utils import run_bass_kernel_spmd

    q = np.ascontiguousarray(np.asarray(q, np.float32))
    k = np.ascontiguousarray(np.asarray(k, np.float32))
    v = np.ascontiguousarray(np.asarray(v, np.float32))
    lam1 = np.exp(np.sum(np.asarray(lambda_q1, np.float32)
                         * np.asarray(lambda_k1, np.float32), dtype=np.float32))
    lam2 = np.exp(np.sum(np.asarray(lambda_q2, np.float32)
                         * np.asarray(lambda_k2, np.float32), dtype=np.float32))
    lam_full = np.float32(lam1 - lam2 + np.float32(LAMBDA_INIT))

    if "nc" not in _CACHE:
        _CACHE["nc"] = build_nc()
    nc = _CACHE["nc"]

    in_maps = _prep_core_inputs(q, k, v, lam_full)
    trace = bool(int(os.environ.get("KERNEL_TRACE", "0")))
    kw = {}
    if trace:
        kw = dict(trace=True, trace_cores=list(range(N_CORES)))
    res = run_bass_kernel_spmd(nc, in_maps, core_ids=list(range(N_CORES)), **kw)
    last_results = res

    out = np.empty((B, S, N_HEADS // 2, 256), np.float32)
    for c in range(N_CORES):
        b = c // 4
        gp = 2 * (c % 4)
        # o: [pair, qb, 128, 4, 256] bf16; row s = qb*512 + j*128 + p
        oc = res.results[c]["o"].astype(np.float32)
        oc = oc.transpose(0, 1, 3, 2, 4).reshape(2, S, 256)
        out[b, :, gp, :] = oc[0]
        out[b, :, gp + 1, :] = oc[1]
    out *= np.asarray(subln_weight, np.float32)[None, None, None, :]
    return out


# revision 37
# speedup vs baseline: 1.0392x; 1.0172x over previous
"""Trainium2 Bass kernel for DiffAttention (nn_DiffAttention_49847390437777).

Contract: kernel(**full_inputs) -> full output [2, 2048, 8, 256] fp32.

Sharding (8 cores): core c handles batch b = c//4 and global query-head pairs
{2*(c%4), 2*(c%4)+1} (i.e. heads 4*(c%4)..4*(c%4)+3).  Diff-attention couples
only adjacent head pairs, which stay co-located.  subln_weight is applied on
host after the gather (it multiplies AFTER the RMS norm, so this is exact).

Device algorithm per core (4 heads = 2 pairs, seq 2048, head_dim 128), all
bf16 matmul inputs (numpy study: all-bf16 rel err ~0.008 vs gate 2e-2; the
old fp32 first-superblock path was unnecessary):
  - scores transposed: S^T[k, q] = kT_blk.T @ qT_blk (contraction d=128 on
    partitions), causal blocks only; softmax without max-subtraction so the
    row-sum fuses into the PV matmul via an extra column on V.
  - exp on ACT per 2-kb group [128, 2, 512] (per-kb trimmed in the diagonal
    region); causal diagonal 128x128 blocks masked with a triangular tile on
    GpSimd (DVE for the first two steps, where GpSimd latency is exposed).
  - PV per 128-row j-block: par0 streams [v1|v2|c0], par1 streams
    [-v1|-v2|c1].  The bf16 V bytes are shared (negated) between the two
    heads so quantization noise cancels in the subtraction (an independently
    rounded lambda*V copy costs 10x accuracy - measured).  c0, c1 is a
    host-searched bf16 pair with c0/c1 ~= lambda to ~1e-5, so the
    normalize-and-subtract needs NO lambda multiply on device:
      A1' = O1 * recip(c0*rowsum1)                   (recip + tensor_scalar)
      diff' = (O2neg * recip(c1*rowsum2)) + A1'      (recip + one STT)
            = (A1 - lambda*A2)/c0
    The 1/c0 scale folds exactly into the RMS norm: eps' = eps/c0^2 is
    shipped via the aux tensor, and rms = exp(-0.5*ln(ssq/256 + eps') +
    ln(1-lambda_init)) then reproduces the reference output bit-for-near.
    Exp+Ln pinned to the one ACT table set holding both (no table thrash).
  - ssq via tensor_mul + reduce_sum, emitted one j late so they queue behind
    the next j's recip chain on the in-order DVE FIFO (PSUM recycling for
    the PV accumulators is gated by that chain).
  - flat software pipeline over (pair, qb) steps crossing the pair boundary:
    scores/exp/mask of step s+1 are emitted before PV+epilogue of step s, so
    exp(pair1,qb0) hides under PV(pair0,qb3).
  - per-half-qb rms/out-scale/store (bf16 output, un-cast on host) so the
    tail only waits on the last two j-blocks; loads and stores ride the
    otherwise-idle sync queue, loads chunked in need-order so the first QK
    starts ~1.5us after DMA go and PV(qb0) is not gated by the V load.
"""

import math
import os

import numpy as np
import ml_dtypes

HEAD_DIM = 128
N_HEADS = 16
LAYER_IDX = 12
LAMBDA_INIT = 0.8 - 0.6 * math.exp(-0.3 * (LAYER_IDX - 1))
EPS = 1e-5
SCALE = 1.0 / math.sqrt(HEAD_DIM)
S_FOLD = 1.0 - LAMBDA_INIT

B = 2
S = 2048
NB = S // 128   # 16 key blocks of 128
QB = S // 512   # 4 query superblocks of 512
N_CORES = 8

bf16 = ml_dtypes.bfloat16

_CACHE = {}
last_results = None  # BassKernelResults of the most recent run (for test.py)


def build_nc():
    """Build + compile the per-core Bass program (same program on all cores)."""
    import concourse.bass as bass
    import concourse.mybir as mybir
    import concourse.bacc as bacc
    import concourse.tile as tile
    from concourse.masks import make_upper_triangular
    from contextlib import ExitStack

    f32 = mybir.dt.float32
    b16 = mybir.dt.bfloat16
    AF = mybir.ActivationFunctionType
    ALU = mybir.AluOpType

    nc = bacc.Bacc("TRN2", target_bir_lowering=False, debug=False)

    # qkb layout per (pair, par): [0:512]=kT[:,0:512], [512:1024]=qT[:,0:512],
    # [1024:1536]=kT[:,512:1024], [1536:2048]=qT[:,512:1024],
    # [2048:3072]=kT[:,1024:2048], [3072:4096]=qT[:,1024:2048];
    # DMA'd in need-order: qb0-chunk, vx par0, qb1-chunk, vx par1, rest.
    # vxb per par: par0=[v1|v2|c0], par1=[-v1|-v2|c1] (see module docstring);
    # aux col0 carries eps/c0^2 for the rms fold.
    qkb = nc.dram_tensor("qkb", [2, 2, 128, 4096], b16, kind="ExternalInput")
    vxb = nc.dram_tensor("vxb", [2, 2, 128, NB, 257], b16, kind="ExternalInput")
    aux = nc.dram_tensor("aux", [128, 2], f32, kind="ExternalInput")
    o = nc.dram_tensor("o", [2, QB, 128, 4, 256], b16, kind="ExternalOutput")

    with tile.TileContext(nc) as tc:
        with ExitStack() as ctx:
            ec = ctx.enter_context
            const = ec(tc.tile_pool(name="const", bufs=1))
            qkpool = ec(tc.tile_pool(name="qkpool", bufs=2))
            vpool = ec(tc.tile_pool(name="vpool", bufs=2))
            ppool = ec(tc.tile_pool(name="ppool", bufs=2))
            apool = ec(tc.tile_pool(name="apool", bufs=2))
            dpool = ec(tc.tile_pool(name="dpool", bufs=2))
            stat = ec(tc.tile_pool(name="stat", bufs=3))
            tmp = ec(tc.tile_pool(name="tmp", bufs=4))
            opool = ec(tc.tile_pool(name="opool", bufs=2))
            spsum = ec(tc.tile_pool(name="spsum", bufs=2, space="PSUM"))
            opsum = ec(tc.tile_pool(name="opsum", bufs=2, space="PSUM"))

            tri16 = const.tile([128, 128], b16)
            make_upper_triangular(nc, tri16[:], val=1.0, diag=True)
            lsf_t = const.tile([128, 1], f32)
            nc.gpsimd.memset(lsf_t[:], math.log(S_FOLD))

            # loads all on the sync queue, in need-order; stores also on sync
            # (it is otherwise idle, and load issues all drain up-front)
            pairdat = {}
            for pair in range(2):
                qk = {}
                for par in range(2):
                    qk[par] = qkpool.tile([128, 4096], b16, tag=f"qk{par}",
                                          name=f"qk{par}")
                vx_b = vpool.tile([128, 2, NB, 257], b16, tag="vx", name="vx")
                for par in range(2):
                    nc.sync.dma_start(qk[par][:, 0:1024],
                                      qkb[pair, par, :, 0:1024])
                nc.sync.dma_start(vx_b[:, 0], vxb[pair, 0])
                nc.sync.dma_start(qk[0][:, 1024:2048],
                                  qkb[pair, 0, :, 1024:2048])
                nc.sync.dma_start(vx_b[:, 1], vxb[pair, 1])
                nc.sync.dma_start(qk[1][:, 1024:2048],
                                  qkb[pair, 1, :, 1024:2048])
                for par in range(2):
                    nc.sync.dma_start(qk[par][:, 2048:4096],
                                      qkb[pair, par, :, 2048:4096])
                pairdat[pair] = (qk, vx_b)
            aux_t = const.tile([128, 2], f32)
            nc.sync.dma_start(aux_t[:], aux[:])
            eps_t = aux_t[:, 0:1]  # eps/c0^2 (c0 folds out in the rms)

            def kt_ap(qk, par, kb):
                t = qk[par]
                if kb < 4:
                    return t[:, kb * 128:(kb + 1) * 128]
                if kb < 8:
                    return t[:, 1024 + (kb - 4) * 128:1024 + (kb - 3) * 128]
                return t[:, 2048 + (kb - 8) * 128:2048 + (kb - 7) * 128]

            def qt_ap(qk, par, qb, qoff):
                t = qk[par]
                if qb == 0:
                    return t[:, 512 + qoff:1024]
                if qb == 1:
                    return t[:, 1536 + qoff:2048]
                base = 3072 + (qb - 2) * 512
                return t[:, base + qoff:base + 512]

            # flat software pipeline over (pair, qb) steps, crossing the pair
            # boundary: scores/exp/mask for step s+1 are emitted before the
            # PV+epilogue of step s, so exp(pair1,qb0) hides under PV(pair0,qb3)
            prev = None  # (pair, qb, {par: pt tile}, vx_b)
            for step in range(2 * QB + 1):
                if step < 2 * QB:
                    pair, qb = divmod(step, QB)
                    qk, vx_b = pairdat[pair]
                    nkb = 4 * qb + 4
                    cur = {}
                    for par in range(2):
                        p1 = ppool.tile([128, NB, 512], b16,
                                        tag=f"pt{par}", name=f"pt{par}")
                        cur[par] = p1
                        for g in range(nkb // 2):
                            sp = spsum.tile([128, 2, 512], f32, tag="sp")
                            for t in range(2):
                                kb = 2 * g + t
                                qoff = max(0, (kb - 4 * qb)) * 128
                                nc.tensor.matmul(
                                    sp[:, t, qoff:512],
                                    kt_ap(qk, par, kb),
                                    qt_ap(qk, par, qb, qoff),
                                    start=True, stop=True,
                                )
                            if 2 * g + 1 < 4 * qb:
                                nc.scalar.activation(
                                    p1[:, 2 * g:2 * g + 2, :], sp[:, :, :],
                                    AF.Exp, scale=SCALE,
                                )
                            else:
                                for t in range(2):
                                    kb = 2 * g + t
                                    qoff = max(0, (kb - 4 * qb)) * 128
                                    nc.scalar.activation(
                                        p1[:, kb, qoff:512],
                                        sp[:, t, qoff:512],
                                        AF.Exp, scale=SCALE,
                                    )
                            mask_eng = nc.vector if step <= 1 else nc.gpsimd
                            for t in range(2):
                                kb = 2 * g + t
                                if kb >= 4 * qb:
                                    qoff = (kb - 4 * qb) * 128
                                    mask_eng.tensor_mul(
                                        p1[:, kb, qoff:qoff + 128],
                                        p1[:, kb, qoff:qoff + 128],
                                        tri16[:],
                                    )
                    nxt = (pair, qb, cur, vx_b)
                else:
                    nxt = None
                if prev is not None:
                    ppair, pqb, ppt, pvx = prev
                    A1q = apool.tile([128, 4, 256], f32, tag="A1", name="A1q")
                    diffq = dpool.tile([128, 4, 256], b16, tag="diff",
                                       name="diffq")
                    ssq = stat.tile([128, 4], f32, tag="ssq", name="ssq")
                    rmst = stat.tile([128, 4], f32, tag="rms", name="rmst")
                    otile = opool.tile([128, 4, 256], b16, tag="ot",
                                       name="otile")
                    def emit_sqred(jj):
                        sqt = tmp.tile([128, 256], b16, tag="sqt")
                        nc.vector.tensor_mul(
                            sqt[:], diffq[:, jj, :], diffq[:, jj, :])
                        nc.vector.reduce_sum(
                            ssq[:, jj:jj + 1], sqt[:],
                            axis=mybir.AxisListType.X)

                    def emit_rms(jj):
                        h0 = jj - 1
                        lnm = stat.tile([128, 2], f32, tag="lnm", name="lnm")
                        nc.scalar.activation(
                            lnm[:], ssq[:, h0:jj + 1], AF.Ln,
                            scale=1.0 / 256.0, bias=eps_t[:])
                        nc.scalar.activation(
                            rmst[:, h0:jj + 1], lnm[:], AF.Exp,
                            scale=-0.5, bias=lsf_t[:])
                        for ji in (h0, jj):
                            nc.vector.tensor_scalar_mul(
                                otile[:, ji, :], diffq[:, ji, :],
                                rmst[:, ji:ji + 1])
                        nc.sync.dma_start(
                            o[ppair, pqb, :, h0:jj + 1],
                            otile[:, h0:jj + 1, :])

                    pend = None
                    for j in range(4):
                        jabs = 4 * pqb + j
                        ops = {}
                        for par in (0, 1):
                            op_t = opsum.tile([128, 257], f32,
                                              tag=f"op{par}")
                            ops[par] = op_t
                            for kb in range(jabs + 1):
                                nc.tensor.matmul(
                                    op_t[:],
                                    ppt[par][:, kb, j * 128:(j + 1) * 128],
                                    pvx[:, par, kb, :],
                                    start=(kb == 0), stop=(kb == jabs),
                                )
                            if par == 0:
                                rc1 = tmp.tile([128, 1], f32, tag="rc1")
                                nc.vector.reciprocal(
                                    rc1[:], op_t[:, 256:257])
                                nc.vector.tensor_scalar_mul(
                                    A1q[:, j, :], op_t[:, 0:256], rc1[:])
                        op_t = ops[1]
                        # par1 streams [-v1|-v2|c1] with c0/c1 ~ lambda to
                        # ~1e-5 (host-searched bf16 pair), so the fused
                        # normalize-and-subtract needs no lambda multiply:
                        # diff' = O1/(c0 r1) - O2/(c1 r2) = (A1 - lam*A2)/c0
                        rcl = tmp.tile([128, 1], f32, tag="rcl")
                        nc.vector.reciprocal(rcl[:], op_t[:, 256:257])
                        nc.vector.scalar_tensor_tensor(
                            diffq[:, j, :], op_t[:, 0:256], rcl[:],
                            A1q[:, j, :], ALU.mult, ALU.add)
                        # sq/reduce of the PREVIOUS j: behind this j's recip
                        # chain in the DVE FIFO, so PSUM recycling isn't
                        # delayed; rms fires once its pair of ssq is in
                        if step == 2 * QB:
                            emit_sqred(j)
                            if j in (1, 3):
                                emit_rms(j)
                        else:
                            if pend is not None:
                                emit_sqred(pend)
                                if pend == 1:
                                    emit_rms(1)
                            pend = j
                    if step != 2 * QB:
                        emit_sqred(3)
                        emit_rms(3)
                prev = nxt

    # Pin Exp+Ln to the one table set containing both
    # (natural_log_exp_and_others) — the greedy per-function chooser otherwise
    # thrashes between exp_and_others and the ln set (~1.3us per reload, and it
    # serializes the pipeline around each switch).
    AF = mybir.ActivationFunctionType
    _orig_gat = bacc.get_activation_tables

    def _gat(arch):
        tabs = _orig_gat(arch)
        for name, fns in tabs.items():
            if name != "natural_log_exp_and_others":
                fns.discard(AF.Exp)
                fns.discard(AF.Ln)
        return tabs

    bacc.get_activation_tables = _gat
    try:
        nc.compile()
    finally:
        bacc.get_activation_tables = _orig_gat
    return nc


def _find_c0c1(lam):
    """bf16 pair (c0, c1) with c0/c1 ~= lam to ~1e-5 (both exactly
    representable, so the ones-columns carry lambda with no bf16 bias)."""
    best = None
    for m in range(256):
        c0 = float(np.float32(bf16(0.5 * (1.0 + m / 256.0))))
        c1 = float(np.float32(bf16(c0 / lam)))
        if c1 <= 0:
            continue
        bias = abs(c0 / (c1 * lam) - 1.0)
        if best is None or bias < best[0]:
            best = (bias, c0, c1)
    return best[1], best[2]


def _prep_core_inputs(q, k, v, lam_full):
    """Host-side shard + layout prep. Returns list of 8 per-core input dicts."""
    c0, c1 = _find_c0c1(float(lam_full))
    aux_ = np.zeros((128, 2), np.float32)
    aux_[:, 0] = EPS / (c0 * c0)
    in_maps = []
    for c in range(N_CORES):
        b = c // 4
        h0 = 4 * (c % 4)
        # [s, 4, d] -> [4, d, s]
        qs = np.ascontiguousarray(q[b, :, h0:h0 + 4, :].transpose(1, 2, 0))
        ks = np.ascontiguousarray(k[b, :, h0:h0 + 4, :].transpose(1, 2, 0))
        # qkb: [pair, par, p, k512|q512|k512|q512|k1024|q1024] bf16
        qkb_ = np.empty((2, 2, 128, 4096), bf16)
        for pair in range(2):
            for par in range(2):
                h = 2 * pair + par
                qkb_[pair, par, :, 0:512] = ks[h][:, 0:512].astype(bf16)
                qkb_[pair, par, :, 512:1024] = qs[h][:, 0:512].astype(bf16)
                qkb_[pair, par, :, 1024:1536] = ks[h][:, 512:1024].astype(bf16)
                qkb_[pair, par, :, 1536:2048] = qs[h][:, 512:1024].astype(bf16)
                qkb_[pair, par, :, 2048:3072] = ks[h][:, 1024:2048].astype(bf16)
                qkb_[pair, par, :, 3072:4096] = qs[h][:, 1024:2048].astype(bf16)
        # par0 = [v1|v2|c0]; par1 = [-v1|-v2|c1].  bf16(-x) == -bf16(x), so
        # the V quantization noise stays perfectly (anti-)correlated between
        # the two heads and cancels in the diff subtraction; c0/c1 carries
        # lambda with ~1e-5 bias since both are exactly representable.
        vx = np.empty((2, 2, S, 257), np.float32)
        for pair in range(2):
            v1 = v[b, :, h0 + 2 * pair, :]
            v2 = v[b, :, h0 + 2 * pair + 1, :]
            vx[pair, 0, :, :128] = v1
            vx[pair, 0, :, 128:256] = v2
            vx[pair, 0, :, 256] = c0
            vx[pair, 1, :, :128] = -v1
            vx[pair, 1, :, 128:256] = -v2
            vx[pair, 1, :, 256] = c1
        # [2, 2, s, 257] -> partition-major [2, 2, 128, nb, 257]
        vxp = vx.reshape(2, 2, NB, 128, 257).transpose(0, 1, 3, 2, 4)
        vxb_ = np.ascontiguousarray(vxp).astype(bf16)
        in_maps.append({"qkb": qkb_, "vxb": vxb_, "aux": aux_})
    return in_maps


def kernel(q, k, v, lambda_q1, lambda_k1, lambda_q2, lambda_k2,
           subln_weight, attention_mask):
    global last_results
    from concourse.bass_utils import run_bass_kernel_spmd

    q = np.ascontiguousarray(np.asarray(q, np.float32))
    k = np.ascontiguousarray(np.asarray(k, np.float32))
    v = np.ascontiguousarray(np.asarray(v, np.float32))
    lam1 = np.exp(np.sum(np.asarray(lambda_q1, np.float32)
                         * np.asarray(lambda_k1, np.float32), dtype=np.float32))
    lam2 = np.exp(np.sum(np.asarray(lambda_q2, np.float32)
                         * np.asarray(lambda_k2, np.float32), dtype=np.float32))
    lam_full = np.float32(lam1 - lam2 + np.float32(LAMBDA_INIT))

    if "nc" not in _CACHE:
        _CACHE["nc"] = build_nc()
    nc = _CACHE["nc"]

    in_maps = _prep_core_inputs(q, k, v, lam_full)
    trace = bool(int(os.environ.get("KERNEL_TRACE", "0")))
    kw = {}
    if trace:
        kw = dict(trace=True, trace_cores=list(range(N_CORES)))
    res = run_bass_kernel_spmd(nc, in_maps, core_ids=list(range(N_CORES)), **kw)
    last_results = res

    out = np.empty((B, S, N_HEADS // 2, 256), np.float32)
    for c in range(N_CORES):
        b = c // 4
        gp = 2 * (c % 4)
        # o: [pair, qb, 128, 4, 256] bf16; row s = qb*512 + j*128 + p
        oc = res.results[c]["o"].astype(np.float32)
        oc = oc.transpose(0, 1, 3, 2, 4).reshape(2, S, 256)
        out[b, :, gp, :] = oc[0]
        out[b, :, gp + 1, :] = oc[1]
    out *= np.asarray(subln_weight, np.float32)[None, None, None, :]
    return out
